# revision 1
# baseline (speedup 1.0000x reference)
"""Equivariant message-passing GNN kernel for 8 Trainium2 NeuronCores.

Strategy (see inline comments):
- atoms are partitioned contiguously across the 8 cores (1250 each); edges are
  routed to their center atom's core and sorted by center.
- per core, atoms are greedily packed into "blocks" of <=25 atom slots whose
  edges fit in 3 tiles of 128 edge slots (padded).
- per edge tile the device: dma_gathers packed neighbor rows (fe/fo, all l,
  3KB/row), multiplies by host-precomputed radial weights (rad), builds a
  block-diagonal "one-hot x C" stationary operand with scalar_tensor_tensor
  (C[e,n,p] = sum_m U[m,n,p] sh[e,m], host-precomputed), and contracts
  edges (x n) on the tensor engine into per-block PSUM accumulators
  pe[(slot,p), (parity,k)].
- per block: PE transpose, per-l linear (block-diag [[We,0],[0,Wo]]) and
  residual add; outputs written in slot space, un-slotted on host.
"""
import numpy as np

import concourse.bass as bass
import concourse.tile as tile
import concourse.mybir as mybir
from concourse import bacc

f32 = mybir.dt.float32
i16 = mybir.dt.int16
MUL = mybir.AluOpType.mult
ADD = mybir.AluOpType.add
EQ = mybir.AluOpType.is_equal

# problem constants (hardcoded per contract)
N_ATOMS = 10000
N_EDGES = 160000
K_L = [64, 48, 32]
NL = [1, 3, 5]
N_BASIS = 16
CUTOFF = 5.0
MP_SCALING = 0.1
N_CORES = 8
A_PER_CORE = N_ATOMS // N_CORES
SLOTS = 25
TILES_PER_BLOCK = 3
EDGE_CAP = TILES_PER_BLOCK * 128
ROW_W = 768
G = 6                                  # tiles per dma_gather
ROWS_L = [SLOTS * NL[l] for l in range(3)]
COLS_L = [2 * K_L[l] for l in range(3)]
GOFF = [(0, 64), (128, 272), (416, 576)]   # table row offsets (A=pe-path, B=po-path)
RAD_OFF = [0, 64, 112]
C_OFF = [0, 1, 10]
C_W = 35
GROFF = [0, 128, 416]
LH_OFF = [850, 625, 0]
LH_W = 875


# ---------------------------------------------------------------- host prep --
def _pack_table(fe, fo):
    t = np.zeros((N_ATOMS, ROW_W), np.float32)
    t[:, 0:64] = fe[0][:, 0, :]
    t[:, 64:128] = fo[0][:, 0, :]
    t[:, 128:272] = fo[1].reshape(N_ATOMS, -1)   # l=1 parity swap (odd l)
    t[:, 272:416] = fe[1].reshape(N_ATOMS, -1)
    t[:, 416:576] = fe[2].reshape(N_ATOMS, -1)
    t[:, 576:736] = fo[2].reshape(N_ATOMS, -1)
    return t


def _host_rad(r, Wr):
    rr = r * CUTOFF
    fcut = 0.5 * (np.cos(np.pi * rr / CUTOFF) + 1.0)
    basis = np.cos(rr[:, None] * np.arange(N_BASIS, dtype=np.float32)) * fcut[:, None]
    return np.concatenate([basis @ Wr[l] for l in range(3)], axis=1).astype(np.float32)


def _host_C(sh, U):
    out = np.zeros((N_EDGES, C_W), np.float32)
    for l in range(3):
        c = np.einsum("em,mnp->enp", sh[l], U[l]) * MP_SCALING
        out[:, C_OFF[l]:C_OFF[l] + NL[l] ** 2] = c.reshape(N_EDGES, -1)
    return out


def _greedy_blocks(edge_counts):
    blocks = []
    i, n = 0, len(edge_counts)
    while i < n:
        ec, na = 0, 0
        a0 = i
        while i < n and na < SLOTS and ec + edge_counts[i] <= EDGE_CAP:
            ec += edge_counts[i]
            na += 1
            i += 1
        if na == 0:
            raise RuntimeError("atom with too many edges for one block")
        blocks.append((a0, na))
    return blocks


def _make_smap():
    s = np.zeros(LH_W, np.float32)
    j, a, p = np.meshgrid(np.arange(5), np.arange(25), np.arange(5), indexing="ij")
    s[0:625] = a.reshape(-1)
    j, a, p = np.meshgrid(np.arange(3), np.arange(25), np.arange(3), indexing="ij")
    s[625:850] = a.reshape(-1)
    s[850:875] = np.arange(25)
    return s


def _prepare(inputs):
    r = np.asarray(inputs["r"], np.float32)
    sh = [np.asarray(inputs[f"sh{l}"], np.float32) for l in range(3)]
    centers = np.asarray(inputs["centers"], np.int64)
    neighbors = np.asarray(inputs["neighbors"], np.int64)
    fe = [np.asarray(inputs[f"fe{l}"], np.float32) for l in range(3)]
    fo = [np.asarray(inputs[f"fo{l}"], np.float32) for l in range(3)]
    Wr = [np.asarray(inputs[f"Wr{l}"], np.float32) for l in range(3)]
    U = [np.asarray(inputs[f"U{l}"], np.float32) for l in range(3)]

    rad_all = _host_rad(r, Wr)
    C_all = _host_C(sh, U)
    table = _pack_table(fe, fo)

    core_of_edge = centers // A_PER_CORE
    percore = []
    for c in range(N_CORES):
        em = np.nonzero(core_of_edge == c)[0]
        order = np.argsort(centers[em], kind="stable")
        eidx = em[order]
        local_atom = centers[eidx] - c * A_PER_CORE
        counts = np.bincount(local_atom, minlength=A_PER_CORE)
        percore.append((eidx, counts, _greedy_blocks(counts)))
    NB = max(len(p[2]) for p in percore)
    NB = ((NB + 1) // 2) * 2
    NT = NB * TILES_PER_BLOCK
    NEP = NT * 128
    NG = NT // G

    meta = {"NB": NB, "NT": NT, "NG": NG, "slots": []}
    cores = []
    for c in range(N_CORES):
        eidx, counts, blk = percore[c]
        nbr = np.zeros(NEP, np.int64)
        radp = np.zeros((NEP, 144), np.float32)
        Cp = np.zeros((NEP, C_W), np.float32)
        arel = np.full(NEP, -1.0, np.float32)
        atom_of_slot = np.full((NB, SLOTS), -1, np.int64)
        starts = np.zeros(A_PER_CORE + 1, np.int64)
        np.cumsum(counts, out=starts[1:])
        for b, (a0, na) in enumerate(blk):
            base = b * EDGE_CAP
            pos = 0
            for s in range(na):
                a = a0 + s
                atom_of_slot[b, s] = a
                lo, hi = starts[a], starts[a + 1]
                seg = eidx[lo:hi]
                sl = slice(base + pos, base + pos + (hi - lo))
                nbr[sl] = neighbors[seg]
                radp[sl] = rad_all[seg]
                Cp[sl] = C_all[seg]
                arel[sl] = float(s)
                pos += hi - lo

        def glay(x):
            w = x.shape[1]
            return np.ascontiguousarray(
                x.reshape(NG, G, 128, w).transpose(0, 2, 1, 3)).reshape(NG, 128, G * w)

        gi = nbr.astype(np.int16).reshape(NG, G * 128)
        col = G * 128 // 16
        gidx = np.zeros((NG, 128, col), np.int16)
        for k in range(8):
            gidx[:, 16 * k:16 * (k + 1), :] = gi.reshape(NG, col, 16).transpose(0, 2, 1)
        resid = []
        for l in range(3):
            R = np.zeros((NB, COLS_L[l], ROWS_L[l]), np.float32)
            for b in range(NB):
                for s in range(SLOTS):
                    a = atom_of_slot[b, s]
                    if a < 0:
                        continue
                    ga = a + c * A_PER_CORE
                    R[b, :K_L[l], s * NL[l]:(s + 1) * NL[l]] = fe[l][ga].T
                    R[b, K_L[l]:, s * NL[l]:(s + 1) * NL[l]] = fo[l][ga].T
            resid.append(R)
        cores.append({
            "gidx": gidx, "rad": glay(radp), "C": glay(Cp),
            "arel": glay(arel.reshape(-1, 1)), "table": table,
            "resid0": resid[0], "resid1": resid[1], "resid2": resid[2],
        })
        meta["slots"].append(atom_of_slot)
    return cores, meta


# ------------------------------------------------------------- bass program --
def _v(ap, dims, extra_off=0):
    return bass.AP(ap.tensor, ap.offset + extra_off, [ap.ap[0]] + dims)


def _build_program(NB):
    NT = NB * TILES_PER_BLOCK
    NG = NT // G
    nc = bacc.Bacc("TRN2", target_bir_lowering=False, debug=False,
                   enable_asserts=False, num_devices=N_CORES)

    table_d = nc.dram_tensor("table", [N_ATOMS, ROW_W], f32, kind="ExternalInput")
    gidx_d = nc.dram_tensor("gidx", [NG, 128, G * 128 // 16], i16, kind="ExternalInput")
    rad_d = nc.dram_tensor("rad", [NG, 128, G * 144], f32, kind="ExternalInput")
    C_d = nc.dram_tensor("C", [NG, 128, G * C_W], f32, kind="ExternalInput")
    arel_d = nc.dram_tensor("arel", [NG, 128, G], f32, kind="ExternalInput")
    smap_d = nc.dram_tensor("smap", [128, LH_W], f32, kind="ExternalInput")
    id_d = nc.dram_tensor("identity", [128, 128], f32, kind="ExternalInput")
    W_d = [nc.dram_tensor(f"W{l}", [COLS_L[l], COLS_L[l]], f32, kind="ExternalInput")
           for l in range(3)]
    resid_d = [nc.dram_tensor(f"resid{l}", [NB, COLS_L[l], ROWS_L[l]], f32,
                              kind="ExternalInput") for l in range(3)]
    out_d = [nc.dram_tensor(f"out{l}", [NB, COLS_L[l], ROWS_L[l]], f32,
                            kind="ExternalOutput") for l in range(3)]

    with tile.TileContext(nc) as tc:
        with tc.tile_pool(name="const", bufs=1) as const, \
             tc.tile_pool(name="gath", bufs=2) as gpool, \
             tc.tile_pool(name="edge", bufs=2) as epool, \
             tc.tile_pool(name="work", bufs=3) as wpool, \
             tc.tile_pool(name="post", bufs=2) as ppool, \
             tc.tile_pool(name="blkps", bufs=2, space="PSUM") as bps, \
             tc.tile_pool(name="postps", bufs=2, space="PSUM") as pps:

            smap = const.tile([128, LH_W], f32, tag="smap")
            nc.sync.dma_start(smap[:], smap_d[:])
            ident = const.tile([128, 128], f32, tag="ident")
            nc.sync.dma_start(ident[:], id_d[:])
            W = []
            for l in range(3):
                w = const.tile([COLS_L[l], COLS_L[l]], f32, tag=f"W{l}", name=f"Wc{l}")
                nc.sync.dma_start(w[:], W_d[l][:])
                W.append(w)

            ps_cur = [None, None, None]
            for g in range(NG):
                ix = epool.tile([128, G * 128 // 16], i16, tag="ix")
                nc.sync.dma_start(ix[:], gidx_d[g])
                gt = gpool.tile([128, G, ROW_W], f32, tag="gt")
                nc.gpsimd.dma_gather(out_ap=gt[:], in_ap=table_d[:], idxs_ap=ix[:],
                                     num_idxs=G * 128, num_idxs_reg=G * 128,
                                     elem_size=ROW_W)
                radg = epool.tile([128, G * 144], f32, tag="radg")
                nc.sync.dma_start(radg[:], rad_d[g])
                Cg = epool.tile([128, G * C_W], f32, tag="Cg")
                nc.sync.dma_start(Cg[:], C_d[g])
                arelg = epool.tile([128, G], f32, tag="arelg")
                nc.sync.dma_start(arelg[:], arel_d[g])

                for tt in range(G):
                    tile_id = g * G + tt
                    b, t = divmod(tile_id, TILES_PER_BLOCK)
                    if t == 0:
                        ps_cur = [bps.tile([ROWS_L[l], COLS_L[l]], f32,
                                           tag=f"ps{l}", name=f"ps{l}_{b}")
                                  for l in range(3)]
                    gr = wpool.tile([128, 736], f32, tag="gr")
                    for l in range(3):
                        K, n = K_L[l], NL[l]
                        for par in range(2):
                            src = gt[:, tt, GOFF[l][par]:GOFF[l][par] + n * K]
                            dst = gr[:, GROFF[l] + par * K:]
                            radl = radg[:, tt * 144 + RAD_OFF[l]:]
                            if n == 1:
                                nc.vector.tensor_tensor(
                                    gr[:, par * K:(par + 1) * K], src,
                                    radg[:, tt * 144:tt * 144 + K], op=MUL)
                            else:
                                nc.vector.tensor_tensor(
                                    _v(dst, [[2 * K, n], [1, K]]),
                                    _v(src, [[K, n], [1, K]]),
                                    _v(radl, [[0, n], [1, K]]), op=MUL)
                    lh = wpool.tile([128, LH_W], f32, tag="lh")
                    ar = arelg[:, tt:tt + 1]
                    for l in range(3):
                        n = NL[l]
                        for j in range(n):
                            off = LH_OFF[l] + j * SLOTS * n
                            o = lh[:, off:off + SLOTS * n]
                            s = smap[:, off:off + SLOTS * n]
                            cj = Cg[:, tt * C_W + C_OFF[l] + j * n:]
                            if n == 1:
                                nc.vector.scalar_tensor_tensor(
                                    o, s, ar, _v(cj, [[0, SLOTS]]), op0=EQ, op1=MUL)
                            else:
                                nc.vector.scalar_tensor_tensor(
                                    _v(o, [[n, SLOTS], [1, n]]),
                                    _v(s, [[n, SLOTS], [1, n]]), ar,
                                    _v(cj, [[0, SLOTS], [1, n]]), op0=EQ, op1=MUL)
                    for l in range(3):
                        K, n = K_L[l], NL[l]
                        for j in range(n):
                            nc.tensor.matmul(
                                ps_cur[l][:],
                                lh[:, LH_OFF[l] + j * SLOTS * n:
                                   LH_OFF[l] + (j + 1) * SLOTS * n],
                                gr[:, GROFF[l] + j * 2 * K:GROFF[l] + (j + 1) * 2 * K],
                                start=(t == 0 and j == 0),
                                stop=(t == TILES_PER_BLOCK - 1 and j == n - 1))
                    if t == TILES_PER_BLOCK - 1:
                        for l in range(3):
                            rows, cols = ROWS_L[l], COLS_L[l]
                            cp = ppool.tile([rows, cols], f32, tag=f"cp{l}")
                            nc.scalar.copy(cp[:], ps_cur[l][:])
                            tp = pps.tile([cols, rows], f32, tag="pp")
                            nc.tensor.transpose(tp[:], cp[:], ident[0:rows, 0:rows])
                            tps = ppool.tile([cols, rows], f32, tag=f"tps{l}")
                            nc.scalar.copy(tps[:], tp[:])
                            om = pps.tile([cols, rows], f32, tag="pp")
                            nc.tensor.matmul(om[:], W[l][:], tps[:],
                                             start=True, stop=True)
                            rs = ppool.tile([cols, rows], f32, tag=f"rs{l}")
                            nc.sync.dma_start(rs[:], resid_d[l][b])
                            ob = ppool.tile([cols, rows], f32, tag=f"ob{l}")
                            nc.vector.tensor_tensor(ob[:], om[:], rs[:], op=ADD)
                            nc.sync.dma_start(out_d[l][b], ob[:])
    nc.compile()
    return nc


# -------------------------------------------------------------------- runner --
def _run_spmd(nc, in_maps):
    import jax
    from jax.sharding import Mesh, PartitionSpec, NamedSharding
    from jax.experimental.shard_map import shard_map
    from concourse.bass2jax import (_bass_exec_p, install_neuronx_cc_hook,
                                    partition_id_tensor)

    install_neuronx_cc_hook()
    partition_name = nc.partition_id_tensor.name if nc.partition_id_tensor else None
    in_names, out_names, out_avals, zero_outs = [], [], [], []
    for alloc in nc.m.functions[0].allocations:
        if not isinstance(alloc, mybir.MemoryLocationSet):
            continue
        name = alloc.memorylocations[0].name
        if alloc.kind == "ExternalInput":
            if name != partition_name:
                in_names.append(name)
        elif alloc.kind == "ExternalOutput":
            out_names.append(name)
            shape = tuple(alloc.tensor_shape)
            dtype = mybir.dt.np(alloc.dtype)
            out_avals.append(jax.core.ShapedArray(shape, dtype))
            zero_outs.append(np.zeros(shape, dtype))
    n_params = len(in_names)
    all_in = list(in_names) + list(out_names)
    if partition_name is not None:
        all_in.append(partition_name)

    def _body(*args):
        operands = list(args)
        if partition_name is not None:
            operands.append(partition_id_tensor())
        return tuple(_bass_exec_p.bind(
            *operands, out_avals=tuple(out_avals), in_names=tuple(all_in),
            out_names=tuple(out_names), lowering_input_output_aliases=(),
            sim_require_finite=False, sim_require_nnan=False, nc=nc))

    n = N_CORES
    devices = jax.devices()[:n]
    mesh = Mesh(np.asarray(devices), ("core",))
    specs = (PartitionSpec("core"),)
    fn = jax.jit(shard_map(_body, mesh=mesh,
                           in_specs=specs * (n_params + len(zero_outs)),
                           out_specs=specs * len(out_names), check_rep=False))
    sh = NamedSharding(mesh, PartitionSpec("core"))
    concat_in = [np.concatenate([np.asarray(m[nm]) for m in in_maps], axis=0)
                 for nm in in_names]
    dev_in = [jax.device_put(a, sh) for a in concat_in]
    dev_z = [jax.device_put(np.zeros((n * z.shape[0], *z.shape[1:]), z.dtype), sh)
             for z in zero_outs]
    outs = fn(*dev_in, *dev_z)
    jax.block_until_ready(outs)
    results = []
    for c in range(n):
        m = {}
        for i, nm in enumerate(out_names):
            full = np.asarray(outs[i])
            per = full.shape[0] // n
            m[nm] = full[c * per:(c + 1) * per]
        results.append(m)
    return results


# -------------------------------------------------------------------- kernel --
def kernel(**inputs):
    cores, meta = _prepare(inputs)
    NB = meta["NB"]
    nc = _build_program(NB)

    We = [np.asarray(inputs[f"We{l}"], np.float32) for l in range(3)]
    Wo = [np.asarray(inputs[f"Wo{l}"], np.float32) for l in range(3)]
    Wblk = []
    for l in range(3):
        K = K_L[l]
        W = np.zeros((2 * K, 2 * K), np.float32)
        W[:K, :K] = We[l]
        W[K:, K:] = Wo[l]
        Wblk.append(W)
    smap = np.broadcast_to(_make_smap()[None, :], (128, LH_W)).copy()
    ident = np.eye(128, dtype=np.float32)

    in_maps = []
    for c in range(N_CORES):
        cd = cores[c]
        in_maps.append(dict(
            table=cd["table"], gidx=cd["gidx"], rad=cd["rad"], C=cd["C"],
            arel=cd["arel"], smap=smap, identity=ident,
            W0=Wblk[0], W1=Wblk[1], W2=Wblk[2],
            resid0=cd["resid0"], resid1=cd["resid1"], resid2=cd["resid2"]))

    res = _run_spmd(nc, in_maps)

    fe = [np.asarray(inputs[f"fe{l}"], np.float32) for l in range(3)]
    fo = [np.asarray(inputs[f"fo{l}"], np.float32) for l in range(3)]
    oe = [fe[l].copy() for l in range(3)]
    oo = [fo[l].copy() for l in range(3)]
    for c in range(N_CORES):
        slots = meta["slots"][c]
        for l in range(3):
            K, nl = K_L[l], NL[l]
            O = res[c][f"out{l}"]
            for b in range(NB):
                for s in range(SLOTS):
                    a = slots[b, s]
                    if a < 0:
                        continue
                    ga = a + c * A_PER_CORE
                    blk = O[b, :, s * nl:(s + 1) * nl]
                    oe[l][ga] = blk[:K].T
                    oo[l][ga] = blk[K:].T
    return (oe[0], oo[0], oe[1], oo[1], oe[2], oo[2])
